# revision 21
# baseline (speedup 1.0000x reference)
"""Trainium2 Bass kernel for causal cross-attention with L2-normalized q/k.

Reference computation (B=4, S=2048, E=512, H=8, Dh=64):
    q = k_embed @ Wq.T ; k = x @ Wk.T ; v = x @ Wv.T        (per batch)
    q,k l2-normalized over Dh per head; scores = g * q @ k.T (causal mask)
    out = softmax(scores) @ v

Sharding: 8 cores = 4 batches x 2 head-groups (4 heads each). Each core:
  - all matmul operands in bf16 (PE runs fp32 at 1/4 rate); inputs are
    host-pre-cast, accumulation stays fp32 in PSUM
  - k is NEVER normalized on-chip: scores use raw k^T and the factor
    g/|k_sk| is folded into the exp's per-partition scale (st has sk on
    the partition dim). The norm is computed from the same bf16 k the
    matmul consumes, so normalization is exact wrt quantized k.
  - q l2-normalized in natural layout (per-partition ops on SBUF bf16)
    then PE-transposed into [head_dim, seq] layout as bf16
  - scores computed transposed: st[sk, sq]; the softmax denominator comes
    from a ones-row appended to v; heads processed ONE at a time so the
    score PSUM tiles double-buffer (PE and Act engine overlap): per head
    st 2x[128,1024] (4 banks) + out accum 2x[65,1024] (4 banks) = 8 banks
  - PSUM->SBUF copies of k^T/q^T go on the Act engine (Copy lives in
    every act table - no table thrash); stat chains stay on DVE
  - returns outT [4 heads, 65, S] bf16 (64 rows of unnormalized out^T + the
    softmax denominator row); host divides and reassembles in fp32.
"""

import numpy as np

B, S, E, H = 4, 2048, 512, 8
Dh = 64
NE = E // 128          # 4 contraction chunks
NT = S // 128          # 16 s-tiles
SQB = 1024             # sq block width (2 PSUM banks)
NJ = S // SQB          # 2


def _build(g: float, repeats: int = 1):
    from contextlib import ExitStack

    import concourse.tile as tile
    from concourse import bacc, mybir
    from concourse.masks import make_identity

    f32 = mybir.dt.float32
    bf16 = mybir.dt.bfloat16
    AF = mybir.ActivationFunctionType
    ALU = mybir.AluOpType

    nc = bacc.Bacc("TRN2", target_bir_lowering=False, debug=False)
    xT_d = nc.dram_tensor("xt", [E, S], bf16, kind="ExternalInput")
    keT_d = nc.dram_tensor("ket", [E, S], bf16, kind="ExternalInput")
    wqT_d = nc.dram_tensor("wqt", [E, 256], bf16, kind="ExternalInput")
    wvkT_d = nc.dram_tensor("wvkt", [E, 512], bf16, kind="ExternalInput")
    mask_d = nc.dram_tensor("mask01", [128, 128], bf16, kind="ExternalInput")
    out_d = nc.dram_tensor("outt", [4, 65, S], bf16, kind="ExternalOutput")

    with tile.TileContext(nc) as tc:
     for _rep in range(repeats):
      with ExitStack() as ctx:
        persist = ctx.enter_context(tc.tile_pool(name=f"persist{_rep}", bufs=1))
        ptmp = ctx.enter_context(tc.tile_pool(name=f"ptmp{_rep}", bufs=3))
        ep = ctx.enter_context(tc.tile_pool(name=f"e_pool{_rep}", bufs=3))
        osb = ctx.enter_context(tc.tile_pool(name=f"o_sb{_rep}", bufs=2))

        # ---- input DMAs (x/ke split into halves for earlier start) ----
        wq_sb = persist.tile([128, NE, 256], bf16, tag="wq")
        wvk_sb = persist.tile([128, NE, 512], bf16, tag="wvk")
        nc.sync.dma_start(out=wvk_sb, in_=wvkT_d.rearrange("(c p) m -> p c m", p=128))
        nc.sync.dma_start(out=wq_sb, in_=wqT_d.rearrange("(c p) m -> p c m", p=128))
        mask_sb = persist.tile([128, 128], bf16, tag="mask")
        nc.sync.dma_start(out=mask_sb, in_=mask_d[:, :])

        SH = S // 2
        SQ4 = S // 4
        x_sbs, ke_sbs = [], []
        for ec in range(NE):
            t = persist.tile([128, S], bf16, tag=f"x{ec}", name=f"x{ec}")
            x_sbs.append(t)
        # quarter-split, quarter-major issue order so the first projection
        # tiles' inputs (all chunks, low s) land first
        for qf in range(4):
            for ec in range(NE):
                nc.sync.dma_start(
                    out=x_sbs[ec][:, qf * SQ4:(qf + 1) * SQ4],
                    in_=xT_d[ec * 128:(ec + 1) * 128, qf * SQ4:(qf + 1) * SQ4])
        for ec in range(NE):
            t = persist.tile([128, S], bf16, tag=f"ke{ec}", name=f"ke{ec}")
            for hf in range(2):
                nc.sync.dma_start(
                    out=t[:, hf * SH:(hf + 1) * SH],
                    in_=keT_d[ec * 128:(ec + 1) * 128, hf * SH:(hf + 1) * SH])
            ke_sbs.append(t)

        ident = persist.tile([128, 128], bf16, tag="ident")
        make_identity(nc, ident[:, :])

        qt_sb = persist.tile([128, 2, S], bf16, tag="qt")   # qhat^T per pair
        kt_sb = persist.tile([128, 2, S], bf16, tag="kt")   # raw k^T per pair
        ksc_sb = persist.tile([128, NT, 4], f32, tag="ksc")  # g/|k| per sk
        rqs_sb = persist.tile([128, NT, 4], f32, tag="rqs")  # 1/|q| per sq
        v_sb = persist.tile([128, NT, 4, 65], bf16, tag="v")  # v + ones col
        nc.vector.memset(v_sb[:, :, :, 64], 1.0)
        # bf16 staging that persists so tiles 8-15's transposes can run
        # underneath the (Act-bound) attention phase
        vkna_sb = persist.tile([128, NT, 512], bf16, tag="vkna")
        qna_sb = persist.tile([128, NT, 4, 64], bf16, tag="qna")

        tp_ctx = ExitStack()
        ptp = tp_ctx.enter_context(
            tc.tile_pool(name=f"pt_ps{_rep}", bufs=2, space="PSUM"))
        proj_ctx = ExitStack()
        pnp = proj_ctx.enter_context(
            tc.tile_pool(name=f"pn_ps{_rep}", bufs=3, space="PSUM"))

        inv_g2 = 1.0 / (g * g)

        def finalize_k(st_i, on_act):
            """PE-transpose raw k tile into kt_sb."""
            kna = vkna_sb[:, st_i, 256:512].rearrange(
                "p (h d) -> p h d", h=4)
            for pair in range(2):
                pst = ptp.tile([128, 128], bf16, tag="t_ps", name="t_ps")
                nc.tensor.transpose(
                    pst[:, :], kna[:, 2 * pair:2 * pair + 2, :], ident[:, :])
                dstk = kt_sb[:, pair, st_i * 128:(st_i + 1) * 128]
                if on_act and pair == 0:
                    nc.scalar.copy(dstk, pst[:, :])
                else:
                    nc.vector.tensor_copy(dstk, pst[:, :])

        def finalize_q(st_i, on_act):
            """Normalize q tile (hat = qna * 1/|q|) and transpose into qt_sb."""
            hat = ptmp.tile([128, 4, 64], bf16, tag="hat", name="hat")
            for h4 in range(4):
                nc.vector.tensor_scalar_mul(
                    hat[:, h4, :], qna_sb[:, st_i, h4, :],
                    rqs_sb[:, st_i, h4:h4 + 1])
            for pair in range(2):
                pst = ptp.tile([128, 128], bf16, tag="t_ps", name="t_ps")
                nc.tensor.transpose(
                    pst[:, :], hat[:, 2 * pair:2 * pair + 2, :], ident[:, :])
                dstq = qt_sb[:, pair, st_i * 128:(st_i + 1) * 128]
                if on_act and pair == 0:
                    nc.scalar.copy(dstq, pst[:, :])
                else:
                    nc.vector.tensor_copy(dstq, pst[:, :])

        # ---- phase A: fused v+k projection; k stays RAW (g/|k| folded
        # into the exp scale later); k^T produced by PE transpose ----
        for st_i in range(NT):
            sl = slice(st_i * 128, (st_i + 1) * 128)
            psvk = pnp.tile([128, 512], f32, tag="nat", name="nat")
            for ec in range(NE):
                nc.tensor.matmul(
                    psvk[:, :], lhsT=x_sbs[ec][:, sl], rhs=wvk_sb[:, ec, :],
                    start=(ec == 0), stop=(ec == NE - 1))
            # one bf16 staging copy for v+k; v_sb filled from it (cheap
            # SBUF->SBUF copy), k half doubles as the transpose input
            nc.vector.tensor_copy(vkna_sb[:, st_i, :], psvk[:, :])
            nc.gpsimd.tensor_copy(
                v_sb[:, st_i, :, 0:64],
                vkna_sb[:, st_i, 0:256].rearrange("p (h d) -> p h d", h=4))
            kna = vkna_sb[:, st_i, 256:512].rearrange("p (h d) -> p h d", h=4)
            ksq = ptmp.tile([128, 4, 64], bf16, tag="ksq", name="ksq")
            nc.gpsimd.tensor_mul(ksq[:, :, :], kna[:, :, :], kna[:, :, :])
            ssk = ptmp.tile([128, 4], f32, tag="ssk", name="ssk")
            nc.vector.tensor_reduce(
                ssk[:, :], ksq[:, :, :], axis=mybir.AxisListType.X, op=ALU.add)
            rnk = ptmp.tile([128, 4], f32, tag="rnk", name="rnk")
            # |k|/g = sqrt(ssk/g^2); reciprocal -> g/|k|
            nc.scalar.activation(rnk[:, :], ssk[:, :], AF.Sqrt, scale=inv_g2)
            nc.vector.reciprocal(ksc_sb[:, st_i, :], rnk[:, :])
            if st_i < 8:
                finalize_k(st_i, on_act=True)

        # ---- phase B: q projection + l2 normalize + transpose ----
        for st_i in range(NT):
            sl = slice(st_i * 128, (st_i + 1) * 128)
            psq = pnp.tile([128, 4, 64], f32, tag="nat", name="nat")
            for ec in range(NE):
                nc.tensor.matmul(
                    psq[:, :, :], lhsT=ke_sbs[ec][:, sl], rhs=wq_sb[:, ec, :],
                    start=(ec == 0), stop=(ec == NE - 1))
            nc.scalar.copy(qna_sb[:, st_i, :, :], psq[:, :, :])
            qsq = ptmp.tile([128, 4, 64], bf16, tag="ksq", name="qsq")
            nc.gpsimd.tensor_mul(
                qsq[:, :, :], qna_sb[:, st_i, :, :], qna_sb[:, st_i, :, :])
            ssq = ptmp.tile([128, 4], f32, tag="ssk", name="ssq")
            nc.vector.tensor_reduce(
                ssq[:, :], qsq[:, :, :], axis=mybir.AxisListType.X, op=ALU.add)
            rnq = ptmp.tile([128, 4], f32, tag="rnk", name="rnq")
            nc.scalar.activation(rnq[:, :], ssq[:, :], AF.Sqrt)
            nc.vector.reciprocal(rqs_sb[:, st_i, :], rnq[:, :])
            if st_i < 8:
                finalize_q(st_i, on_act=True)
        proj_ctx.close()

        # ---- attention: one head at a time, transposed scores, ones-row
        # denominator; st double-buffered so exp (Act) overlaps matmuls (PE).
        # j OUTER: the j=0 round only needs tiles 0-7, so tiles 8-15's
        # transposes (PE/DVE-only) hide under the Act-bound j=0 round.
        with tc.tile_pool(name=f"st_ps{_rep}", bufs=2, space="PSUM") as stp, \
             tc.tile_pool(name=f"o_ps{_rep}", bufs=1, space="PSUM") as op:
            for j in range(NJ):
                for h in range(4):
                    pair, hp = h // 2, h % 2
                    psl = slice(hp * 64, (hp + 1) * 64)
                    ops = op.tile([65, SQB], f32, tag="o", name="o")
                    for i in range(8 * j + 8):
                        co = max(0, 128 * i - SQB * j)
                        corner = 128 * i >= SQB * j
                        ksl = slice(i * 128, (i + 1) * 128)
                        st = stp.tile([128, SQB], f32, tag="st", name="st")
                        for c0, c1 in ((co, 512), (max(co, 512), SQB)):
                            if c0 >= c1:
                                continue
                            nc.tensor.matmul(
                                st[:, c0:c1],
                                lhsT=kt_sb[psl, pair, ksl],
                                rhs=qt_sb[psl, pair, j * SQB + c0:j * SQB + c1],
                                start=True, stop=True)
                        et = ep.tile([128, SQB], bf16, tag="e", name="e")
                        nc.scalar.activation(
                            et[:, co:SQB], st[:, co:SQB], AF.Exp,
                            scale=ksc_sb[:, i, h:h + 1])
                        if corner:
                            nc.vector.tensor_mul(
                                et[:, co:co + 128], et[:, co:co + 128],
                                mask_sb[:, :])
                        for a, b in ((co, 512), (max(co, 512), SQB)):
                            if a >= b:
                                continue
                            # last matmul per PSUM bank: bank A's final write
                            # is tile i == 8j+3 (corner), bank B's i == 8j+7
                            is_last = ((i == 8 * j + 3) if b <= 512
                                       else (i == 8 * j + 7))
                            nc.tensor.matmul(
                                ops[:, a:b],
                                lhsT=v_sb[:, i, h, :],
                                rhs=et[:, a:b],
                                start=(i == 0), stop=is_last)
                    ot = osb.tile([65, SQB], bf16, tag="osb", name="osb")
                    nc.vector.tensor_copy(ot[:, :], ops[:, :])
                    nc.sync.dma_start(
                        out_d[h, :, j * SQB:(j + 1) * SQB], ot[:, :])
                    if j == 0:
                        # hide tiles 8-15's transposes under the Act-bound
                        # j=0 round (PE/DVE only - no act-table thrash)
                        for t in (8 + 2 * h, 9 + 2 * h):
                            finalize_k(t, on_act=False)
                            finalize_q(t, on_act=False)
        tp_ctx.close()
    nc.compile()
    return nc


_NC_CACHE = {}


def _get_nc(g: float):
    if g not in _NC_CACHE:
        _NC_CACHE[g] = _build(g)
    return _NC_CACHE[g]


def _numpy_fallback(x, k_embed, attn_mask, key_padding_mask, Wq, Wk, Wv, g_scale):
    def l2n(t):
        n = np.sqrt((t * t).sum(-1, keepdims=True))
        return t / np.maximum(n, 1e-12)
    q = (k_embed @ Wq.T).reshape(B, S, H, Dh).transpose(0, 2, 1, 3)
    k = (x @ Wk.T).reshape(B, S, H, Dh).transpose(0, 2, 1, 3)
    v = (x @ Wv.T).reshape(B, S, H, Dh).transpose(0, 2, 1, 3)
    q, k = l2n(q), l2n(k)
    s = float(g_scale) * np.einsum('bhqd,bhkd->bhqk', q, k)
    s = np.where(attn_mask[None, None], -np.inf, s)
    s = np.where(key_padding_mask[:, None, None, :], -np.inf, s)
    s = s - s.max(-1, keepdims=True)
    e = np.exp(s)
    a = e / e.sum(-1, keepdims=True)
    o = np.einsum('bhqk,bhkd->bhqd', a, v)
    return o.transpose(0, 2, 1, 3).reshape(B, S, E).astype(np.float32)


def _make_in_maps(x, k_embed, Wq, Wk, Wv):
    import ml_dtypes
    bf = ml_dtypes.bfloat16
    mask01 = np.triu(np.ones((128, 128), np.float32)).astype(bf)  # keep sq >= sk
    in_maps = []
    for c in range(8):
        b, hg = c // 2, c % 2
        rows = slice(hg * 256, (hg + 1) * 256)
        wv_t = Wv[rows].T                       # [512, 256]
        wk_t = Wk[rows].T
        in_maps.append({
            "xt": np.ascontiguousarray(x[b].T).astype(bf),
            "ket": np.ascontiguousarray(k_embed[b].T).astype(bf),
            "wqt": np.ascontiguousarray(Wq[rows].T).astype(bf),
            "wvkt": np.ascontiguousarray(
                np.concatenate([wv_t, wk_t], axis=1)).astype(bf),
            "mask01": mask01,
        })
    return in_maps


def kernel(**inputs) -> np.ndarray:
    x = np.asarray(inputs["x"], np.float32)
    k_embed = np.asarray(inputs["k_embed"], np.float32)
    attn_mask = np.asarray(inputs["attn_mask"])
    key_padding_mask = np.asarray(inputs["key_padding_mask"])
    Wq = np.asarray(inputs["Wq"], np.float32)
    Wk = np.asarray(inputs["Wk"], np.float32)
    Wv = np.asarray(inputs["Wv"], np.float32)
    g = float(np.asarray(inputs["g_scale"]))

    causal = np.triu(np.ones((S, S), bool), k=1)
    if (attn_mask != causal).any() or key_padding_mask.any():
        return _numpy_fallback(x, k_embed, attn_mask, key_padding_mask,
                               Wq, Wk, Wv, g)

    from concourse.bass_utils import run_bass_kernel_spmd

    nc = _get_nc(g)
    in_maps = _make_in_maps(x, k_embed, Wq, Wk, Wv)
    res = run_bass_kernel_spmd(nc, in_maps, core_ids=list(range(8)))
    kernel._last_results = res

    out = np.empty((B, S, E), np.float32)
    for c in range(8):
        b, hg = c // 2, c % 2
        r = np.asarray(res.results[c]["outt"], dtype=np.float32)  # [4, 65, S]
        w = r[:, :64, :] / r[:, 64:65, :]   # normalize by softmax denom
        for hh in range(4):
            h = hg * 4 + hh
            out[b, :, h * 64:(h + 1) * 64] = w[hh].T
    return out


# revision 22
# speedup vs baseline: 1.0105x; 1.0105x over previous
"""Trainium2 Bass kernel for causal cross-attention with L2-normalized q/k.

Reference computation (B=4, S=2048, E=512, H=8, Dh=64):
    q = k_embed @ Wq.T ; k = x @ Wk.T ; v = x @ Wv.T        (per batch)
    q,k l2-normalized over Dh per head; scores = g * q @ k.T (causal mask)
    out = softmax(scores) @ v

Sharding: 8 cores = 4 batches x 2 head-groups (4 heads each). Each core:
  - all matmul operands in bf16 (PE runs fp32 at 1/4 rate); inputs are
    host-pre-cast, accumulation stays fp32 in PSUM
  - k is NEVER normalized on-chip: scores use raw k^T and the factor
    g/|k_sk| is folded into the exp's per-partition scale (st has sk on
    the partition dim). The norm is computed from the same bf16 k the
    matmul consumes, so normalization is exact wrt quantized k.
  - q l2-normalized in natural layout (per-partition ops on SBUF bf16)
    then PE-transposed into [head_dim, seq] layout as bf16
  - scores computed transposed: st[sk, sq]; the softmax denominator comes
    from a ones-row appended to v; heads processed ONE at a time so the
    score PSUM tiles double-buffer (PE and Act engine overlap): per head
    st 2x[128,1024] (4 banks) + out accum 2x[65,1024] (4 banks) = 8 banks
  - PSUM->SBUF copies of k^T/q^T go on the Act engine (Copy lives in
    every act table - no table thrash); stat chains stay on DVE
  - returns outT [4 heads, 65, S] bf16 (64 rows of unnormalized out^T + the
    softmax denominator row); host divides and reassembles in fp32.
"""

import numpy as np

B, S, E, H = 4, 2048, 512, 8
Dh = 64
NE = E // 128          # 4 contraction chunks
NT = S // 128          # 16 s-tiles
SQB = 1024             # sq block width (2 PSUM banks)
NJ = S // SQB          # 2


def _build(g: float, repeats: int = 1):
    from contextlib import ExitStack

    import concourse.tile as tile
    from concourse import bacc, mybir
    from concourse.masks import make_identity

    f32 = mybir.dt.float32
    bf16 = mybir.dt.bfloat16
    AF = mybir.ActivationFunctionType
    ALU = mybir.AluOpType

    nc = bacc.Bacc("TRN2", target_bir_lowering=False, debug=False)
    xT_d = nc.dram_tensor("xt", [E, S], bf16, kind="ExternalInput")
    keT_d = nc.dram_tensor("ket", [E, S], bf16, kind="ExternalInput")
    wqT_d = nc.dram_tensor("wqt", [E, 256], bf16, kind="ExternalInput")
    wvkT_d = nc.dram_tensor("wvkt", [E, 512], bf16, kind="ExternalInput")
    mask_d = nc.dram_tensor("mask01", [128, 128], bf16, kind="ExternalInput")
    out_d = nc.dram_tensor("outt", [4, 65, S], bf16, kind="ExternalOutput")

    with tile.TileContext(nc) as tc:
     for _rep in range(repeats):
      with ExitStack() as ctx:
        persist = ctx.enter_context(tc.tile_pool(name=f"persist{_rep}", bufs=1))
        ptmp = ctx.enter_context(tc.tile_pool(name=f"ptmp{_rep}", bufs=3))
        ep = ctx.enter_context(tc.tile_pool(name=f"e_pool{_rep}", bufs=3))
        osb = ctx.enter_context(tc.tile_pool(name=f"o_sb{_rep}", bufs=2))

        # ---- input DMAs (x/ke split into halves for earlier start) ----
        wq_sb = persist.tile([128, NE, 256], bf16, tag="wq")
        wvk_sb = persist.tile([128, NE, 512], bf16, tag="wvk")
        mask_sb = persist.tile([128, 128], bf16, tag="mask")

        SQ4 = S // 4
        x_sbs, ke_sbs = [], []
        for ec in range(NE):
            t = persist.tile([128, S], bf16, tag=f"x{ec}", name=f"x{ec}")
            x_sbs.append(t)
        for ec in range(NE):
            t = persist.tile([128, S], bf16, tag=f"ke{ec}", name=f"ke{ec}")
            ke_sbs.append(t)
        # issue order = consumption order: wvk + x quarter-0 gate the first
        # projection; wq/mask/ke follow; later x/ke quarters last
        nc.sync.dma_start(out=wvk_sb, in_=wvkT_d.rearrange("(c p) m -> p c m", p=128))
        for ec in range(NE):
            nc.sync.dma_start(
                out=x_sbs[ec][:, 0:SQ4],
                in_=xT_d[ec * 128:(ec + 1) * 128, 0:SQ4])
        nc.sync.dma_start(out=wq_sb, in_=wqT_d.rearrange("(c p) m -> p c m", p=128))
        nc.sync.dma_start(out=mask_sb, in_=mask_d[:, :])
        for qf in range(1, 4):
            for ec in range(NE):
                nc.sync.dma_start(
                    out=x_sbs[ec][:, qf * SQ4:(qf + 1) * SQ4],
                    in_=xT_d[ec * 128:(ec + 1) * 128, qf * SQ4:(qf + 1) * SQ4])
        for qf in range(4):
            for ec in range(NE):
                nc.sync.dma_start(
                    out=ke_sbs[ec][:, qf * SQ4:(qf + 1) * SQ4],
                    in_=keT_d[ec * 128:(ec + 1) * 128, qf * SQ4:(qf + 1) * SQ4])

        ident = persist.tile([128, 128], bf16, tag="ident")
        make_identity(nc, ident[:, :])

        qt_sb = persist.tile([128, 2, S], bf16, tag="qt")   # qhat^T per pair
        kt_sb = persist.tile([128, 2, S], bf16, tag="kt")   # raw k^T per pair
        ksc_sb = persist.tile([128, NT, 4], f32, tag="ksc")  # g/|k| per sk
        rqs_sb = persist.tile([128, NT, 4], f32, tag="rqs")  # 1/|q| per sq
        v_sb = persist.tile([128, NT, 4, 65], bf16, tag="v")  # v + ones col
        nc.vector.memset(v_sb[:, :, :, 64], 1.0)
        # bf16 staging that persists so tiles 8-15's transposes can run
        # underneath the (Act-bound) attention phase
        vkna_sb = persist.tile([128, NT, 512], bf16, tag="vkna")
        qna_sb = persist.tile([128, NT, 4, 64], bf16, tag="qna")

        tp_ctx = ExitStack()
        ptp = tp_ctx.enter_context(
            tc.tile_pool(name=f"pt_ps{_rep}", bufs=2, space="PSUM"))
        proj_ctx = ExitStack()
        pnp = proj_ctx.enter_context(
            tc.tile_pool(name=f"pn_ps{_rep}", bufs=3, space="PSUM"))

        inv_g2 = 1.0 / (g * g)

        def finalize_k(st_i, on_act):
            """PE-transpose raw k tile into kt_sb."""
            kna = vkna_sb[:, st_i, 256:512].rearrange(
                "p (h d) -> p h d", h=4)
            for pair in range(2):
                pst = ptp.tile([128, 128], bf16, tag="t_ps", name="t_ps")
                nc.tensor.transpose(
                    pst[:, :], kna[:, 2 * pair:2 * pair + 2, :], ident[:, :])
                dstk = kt_sb[:, pair, st_i * 128:(st_i + 1) * 128]
                if on_act and pair == 0:
                    nc.scalar.copy(dstk, pst[:, :])
                else:
                    nc.vector.tensor_copy(dstk, pst[:, :])

        def finalize_q(st_i, on_act):
            """Normalize q tile (hat = qna * 1/|q|) and transpose into qt_sb."""
            hat = ptmp.tile([128, 4, 64], bf16, tag="hat", name="hat")
            for h4 in range(4):
                nc.vector.tensor_scalar_mul(
                    hat[:, h4, :], qna_sb[:, st_i, h4, :],
                    rqs_sb[:, st_i, h4:h4 + 1])
            for pair in range(2):
                pst = ptp.tile([128, 128], bf16, tag="t_ps", name="t_ps")
                nc.tensor.transpose(
                    pst[:, :], hat[:, 2 * pair:2 * pair + 2, :], ident[:, :])
                dstq = qt_sb[:, pair, st_i * 128:(st_i + 1) * 128]
                if on_act and pair == 0:
                    nc.scalar.copy(dstq, pst[:, :])
                else:
                    nc.vector.tensor_copy(dstq, pst[:, :])

        # ---- phase A: fused v+k projection; k stays RAW (g/|k| folded
        # into the exp scale later); k^T produced by PE transpose ----
        for st_i in range(NT):
            sl = slice(st_i * 128, (st_i + 1) * 128)
            psvk = pnp.tile([128, 512], f32, tag="nat", name="nat")
            for ec in range(NE):
                nc.tensor.matmul(
                    psvk[:, :], lhsT=x_sbs[ec][:, sl], rhs=wvk_sb[:, ec, :],
                    start=(ec == 0), stop=(ec == NE - 1))
            # one bf16 staging copy for v+k; v_sb filled from it (cheap
            # SBUF->SBUF copy), k half doubles as the transpose input
            nc.vector.tensor_copy(vkna_sb[:, st_i, :], psvk[:, :])
            nc.gpsimd.tensor_copy(
                v_sb[:, st_i, :, 0:64],
                vkna_sb[:, st_i, 0:256].rearrange("p (h d) -> p h d", h=4))
            kna = vkna_sb[:, st_i, 256:512].rearrange("p (h d) -> p h d", h=4)
            ksq = ptmp.tile([128, 4, 64], bf16, tag="ksq", name="ksq")
            nc.gpsimd.tensor_mul(ksq[:, :, :], kna[:, :, :], kna[:, :, :])
            ssk = ptmp.tile([128, 4], f32, tag="ssk", name="ssk")
            nc.vector.tensor_reduce(
                ssk[:, :], ksq[:, :, :], axis=mybir.AxisListType.X, op=ALU.add)
            rnk = ptmp.tile([128, 4], f32, tag="rnk", name="rnk")
            # |k|/g = sqrt(ssk/g^2); reciprocal -> g/|k|
            nc.scalar.activation(rnk[:, :], ssk[:, :], AF.Sqrt, scale=inv_g2)
            nc.vector.reciprocal(ksc_sb[:, st_i, :], rnk[:, :])
            if st_i < 8:
                finalize_k(st_i, on_act=True)

        # ---- phase B: q projection + l2 normalize + transpose ----
        for st_i in range(NT):
            sl = slice(st_i * 128, (st_i + 1) * 128)
            psq = pnp.tile([128, 4, 64], f32, tag="nat", name="nat")
            for ec in range(NE):
                nc.tensor.matmul(
                    psq[:, :, :], lhsT=ke_sbs[ec][:, sl], rhs=wq_sb[:, ec, :],
                    start=(ec == 0), stop=(ec == NE - 1))
            nc.scalar.copy(qna_sb[:, st_i, :, :], psq[:, :, :])
            qsq = ptmp.tile([128, 4, 64], bf16, tag="ksq", name="qsq")
            nc.gpsimd.tensor_mul(
                qsq[:, :, :], qna_sb[:, st_i, :, :], qna_sb[:, st_i, :, :])
            ssq = ptmp.tile([128, 4], f32, tag="ssk", name="ssq")
            nc.vector.tensor_reduce(
                ssq[:, :], qsq[:, :, :], axis=mybir.AxisListType.X, op=ALU.add)
            rnq = ptmp.tile([128, 4], f32, tag="rnk", name="rnq")
            nc.scalar.activation(rnq[:, :], ssq[:, :], AF.Sqrt)
            nc.vector.reciprocal(rqs_sb[:, st_i, :], rnq[:, :])
            if st_i < 8:
                finalize_q(st_i, on_act=True)
        proj_ctx.close()

        # ---- attention: one head at a time, transposed scores, ones-row
        # denominator; st double-buffered so exp (Act) overlaps matmuls (PE).
        # j OUTER: the j=0 round only needs tiles 0-7, so tiles 8-15's
        # transposes (PE/DVE-only) hide under the Act-bound j=0 round.
        with tc.tile_pool(name=f"st_ps{_rep}", bufs=2, space="PSUM") as stp, \
             tc.tile_pool(name=f"o_ps{_rep}", bufs=1, space="PSUM") as op:
            for j in range(NJ):
                for h in range(4):
                    pair, hp = h // 2, h % 2
                    psl = slice(hp * 64, (hp + 1) * 64)
                    ops = op.tile([65, SQB], f32, tag="o", name="o")
                    for i in range(8 * j + 8):
                        co = max(0, 128 * i - SQB * j)
                        corner = 128 * i >= SQB * j
                        ksl = slice(i * 128, (i + 1) * 128)
                        st = stp.tile([128, SQB], f32, tag="st", name="st")
                        for c0, c1 in ((co, 512), (max(co, 512), SQB)):
                            if c0 >= c1:
                                continue
                            nc.tensor.matmul(
                                st[:, c0:c1],
                                lhsT=kt_sb[psl, pair, ksl],
                                rhs=qt_sb[psl, pair, j * SQB + c0:j * SQB + c1],
                                start=True, stop=True)
                        et = ep.tile([128, SQB], bf16, tag="e", name="e")
                        nc.scalar.activation(
                            et[:, co:SQB], st[:, co:SQB], AF.Exp,
                            scale=ksc_sb[:, i, h:h + 1])
                        if corner:
                            nc.vector.tensor_mul(
                                et[:, co:co + 128], et[:, co:co + 128],
                                mask_sb[:, :])
                        for a, b in ((co, 512), (max(co, 512), SQB)):
                            if a >= b:
                                continue
                            # last matmul per PSUM bank: bank A's final write
                            # is tile i == 8j+3 (corner), bank B's i == 8j+7
                            is_last = ((i == 8 * j + 3) if b <= 512
                                       else (i == 8 * j + 7))
                            nc.tensor.matmul(
                                ops[:, a:b],
                                lhsT=v_sb[:, i, h, :],
                                rhs=et[:, a:b],
                                start=(i == 0), stop=is_last)
                    ot = osb.tile([65, SQB], bf16, tag="osb", name="osb")
                    nc.vector.tensor_copy(ot[:, :], ops[:, :])
                    nc.sync.dma_start(
                        out_d[h, :, j * SQB:(j + 1) * SQB], ot[:, :])
                    if j == 0:
                        # hide tiles 8-15's transposes under the Act-bound
                        # j=0 round (PE/DVE only - no act-table thrash)
                        for t in (8 + 2 * h, 9 + 2 * h):
                            finalize_k(t, on_act=False)
                            finalize_q(t, on_act=False)
        tp_ctx.close()
    nc.compile()
    return nc


_NC_CACHE = {}


def _get_nc(g: float):
    if g not in _NC_CACHE:
        _NC_CACHE[g] = _build(g)
    return _NC_CACHE[g]


def _numpy_fallback(x, k_embed, attn_mask, key_padding_mask, Wq, Wk, Wv, g_scale):
    def l2n(t):
        n = np.sqrt((t * t).sum(-1, keepdims=True))
        return t / np.maximum(n, 1e-12)
    q = (k_embed @ Wq.T).reshape(B, S, H, Dh).transpose(0, 2, 1, 3)
    k = (x @ Wk.T).reshape(B, S, H, Dh).transpose(0, 2, 1, 3)
    v = (x @ Wv.T).reshape(B, S, H, Dh).transpose(0, 2, 1, 3)
    q, k = l2n(q), l2n(k)
    s = float(g_scale) * np.einsum('bhqd,bhkd->bhqk', q, k)
    s = np.where(attn_mask[None, None], -np.inf, s)
    s = np.where(key_padding_mask[:, None, None, :], -np.inf, s)
    s = s - s.max(-1, keepdims=True)
    e = np.exp(s)
    a = e / e.sum(-1, keepdims=True)
    o = np.einsum('bhqk,bhkd->bhqd', a, v)
    return o.transpose(0, 2, 1, 3).reshape(B, S, E).astype(np.float32)


def _make_in_maps(x, k_embed, Wq, Wk, Wv):
    import ml_dtypes
    bf = ml_dtypes.bfloat16
    mask01 = np.triu(np.ones((128, 128), np.float32)).astype(bf)  # keep sq >= sk
    in_maps = []
    for c in range(8):
        b, hg = c // 2, c % 2
        rows = slice(hg * 256, (hg + 1) * 256)
        wv_t = Wv[rows].T                       # [512, 256]
        wk_t = Wk[rows].T
        in_maps.append({
            "xt": np.ascontiguousarray(x[b].T).astype(bf),
            "ket": np.ascontiguousarray(k_embed[b].T).astype(bf),
            "wqt": np.ascontiguousarray(Wq[rows].T).astype(bf),
            "wvkt": np.ascontiguousarray(
                np.concatenate([wv_t, wk_t], axis=1)).astype(bf),
            "mask01": mask01,
        })
    return in_maps


def kernel(**inputs) -> np.ndarray:
    x = np.asarray(inputs["x"], np.float32)
    k_embed = np.asarray(inputs["k_embed"], np.float32)
    attn_mask = np.asarray(inputs["attn_mask"])
    key_padding_mask = np.asarray(inputs["key_padding_mask"])
    Wq = np.asarray(inputs["Wq"], np.float32)
    Wk = np.asarray(inputs["Wk"], np.float32)
    Wv = np.asarray(inputs["Wv"], np.float32)
    g = float(np.asarray(inputs["g_scale"]))

    causal = np.triu(np.ones((S, S), bool), k=1)
    if (attn_mask != causal).any() or key_padding_mask.any():
        return _numpy_fallback(x, k_embed, attn_mask, key_padding_mask,
                               Wq, Wk, Wv, g)

    from concourse.bass_utils import run_bass_kernel_spmd

    nc = _get_nc(g)
    in_maps = _make_in_maps(x, k_embed, Wq, Wk, Wv)
    res = run_bass_kernel_spmd(nc, in_maps, core_ids=list(range(8)))
    kernel._last_results = res

    out = np.empty((B, S, E), np.float32)
    for c in range(8):
        b, hg = c // 2, c % 2
        r = np.asarray(res.results[c]["outt"], dtype=np.float32)  # [4, 65, S]
        w = r[:, :64, :] / r[:, 64:65, :]   # normalize by softmax denom
        for hh in range(4):
            h = hg * 4 + hh
            out[b, :, h * 64:(h + 1) * 64] = w[hh].T
    return out


# revision 23
# speedup vs baseline: 1.0114x; 1.0009x over previous
"""Trainium2 Bass kernel for causal cross-attention with L2-normalized q/k.

Reference computation (B=4, S=2048, E=512, H=8, Dh=64):
    q = k_embed @ Wq.T ; k = x @ Wk.T ; v = x @ Wv.T        (per batch)
    q,k l2-normalized over Dh per head; scores = g * q @ k.T (causal mask)
    out = softmax(scores) @ v

Sharding: 8 cores = 4 batches x 2 head-groups (4 heads each). Each core:
  - all matmul operands in bf16 (PE runs fp32 at 1/4 rate); inputs are
    host-pre-cast, accumulation stays fp32 in PSUM
  - k is NEVER normalized on-chip: scores use raw k^T and the factor
    g/|k_sk| is folded into the exp's per-partition scale (st has sk on
    the partition dim). The norm is computed from the same bf16 k the
    matmul consumes, so normalization is exact wrt quantized k.
  - q l2-normalized in natural layout (per-partition ops on SBUF bf16)
    then PE-transposed into [head_dim, seq] layout as bf16
  - scores computed transposed: st[sk, sq]; the softmax denominator comes
    from a ones-row appended to v; heads processed ONE at a time so the
    score PSUM tiles double-buffer (PE and Act engine overlap): per head
    st 2x[128,1024] (4 banks) + out accum 2x[65,1024] (4 banks) = 8 banks
  - PSUM->SBUF copies of k^T/q^T go on the Act engine (Copy lives in
    every act table - no table thrash); stat chains stay on DVE
  - returns outT [4 heads, 65, S] bf16 (64 rows of unnormalized out^T + the
    softmax denominator row); host divides and reassembles in fp32.
"""

import numpy as np

B, S, E, H = 4, 2048, 512, 8
Dh = 64
NE = E // 128          # 4 contraction chunks
NT = S // 128          # 16 s-tiles
SQB = 1024             # sq block width (2 PSUM banks)
NJ = S // SQB          # 2


def _build(g: float, repeats: int = 1):
    from contextlib import ExitStack

    import concourse.tile as tile
    from concourse import bacc, mybir
    from concourse.masks import make_identity

    f32 = mybir.dt.float32
    bf16 = mybir.dt.bfloat16
    AF = mybir.ActivationFunctionType
    ALU = mybir.AluOpType

    nc = bacc.Bacc("TRN2", target_bir_lowering=False, debug=False)
    xT_d = nc.dram_tensor("xt", [E, S], bf16, kind="ExternalInput")
    keT_d = nc.dram_tensor("ket", [E, S], bf16, kind="ExternalInput")
    wqT_d = nc.dram_tensor("wqt", [E, 256], bf16, kind="ExternalInput")
    wvkT_d = nc.dram_tensor("wvkt", [E, 512], bf16, kind="ExternalInput")
    mask_d = nc.dram_tensor("mask01", [128, 128], bf16, kind="ExternalInput")
    out_d = nc.dram_tensor("outt", [4, 65, S], bf16, kind="ExternalOutput")

    with tile.TileContext(nc) as tc:
     for _rep in range(repeats):
      with ExitStack() as ctx:
        persist = ctx.enter_context(tc.tile_pool(name=f"persist{_rep}", bufs=1))
        ptmp = ctx.enter_context(tc.tile_pool(name=f"ptmp{_rep}", bufs=3))
        ep = ctx.enter_context(tc.tile_pool(name=f"e_pool{_rep}", bufs=3))
        osb = ctx.enter_context(tc.tile_pool(name=f"o_sb{_rep}", bufs=2))

        # ---- input DMAs (x/ke split into halves for earlier start) ----
        wq_sb = persist.tile([128, NE, 256], bf16, tag="wq")
        wvk_sb = persist.tile([128, NE, 512], bf16, tag="wvk")
        mask_sb = persist.tile([128, 128], bf16, tag="mask")

        SQ4 = S // 4
        x_sbs, ke_sbs = [], []
        for ec in range(NE):
            t = persist.tile([128, S], bf16, tag=f"x{ec}", name=f"x{ec}")
            x_sbs.append(t)
        for ec in range(NE):
            t = persist.tile([128, S], bf16, tag=f"ke{ec}", name=f"ke{ec}")
            ke_sbs.append(t)
        # issue order = consumption order: wvk + x quarter-0 gate the first
        # projection; wq/mask/ke follow; later x/ke quarters last
        nc.sync.dma_start(out=wvk_sb, in_=wvkT_d.rearrange("(c p) m -> p c m", p=128))
        for ec in range(NE):
            nc.sync.dma_start(
                out=x_sbs[ec][:, 0:SQ4],
                in_=xT_d[ec * 128:(ec + 1) * 128, 0:SQ4])
        nc.sync.dma_start(out=wq_sb, in_=wqT_d.rearrange("(c p) m -> p c m", p=128))
        nc.sync.dma_start(out=mask_sb, in_=mask_d[:, :])
        for qf in range(1, 4):
            for ec in range(NE):
                nc.sync.dma_start(
                    out=x_sbs[ec][:, qf * SQ4:(qf + 1) * SQ4],
                    in_=xT_d[ec * 128:(ec + 1) * 128, qf * SQ4:(qf + 1) * SQ4])
        for qf in range(4):
            for ec in range(NE):
                nc.sync.dma_start(
                    out=ke_sbs[ec][:, qf * SQ4:(qf + 1) * SQ4],
                    in_=keT_d[ec * 128:(ec + 1) * 128, qf * SQ4:(qf + 1) * SQ4])

        ident = persist.tile([128, 128], bf16, tag="ident")
        make_identity(nc, ident[:, :])

        qt_sb = persist.tile([128, 2, S], bf16, tag="qt")   # qhat^T per pair
        kt_sb = persist.tile([128, 2, S], bf16, tag="kt")   # raw k^T per pair
        ksc_sb = persist.tile([128, NT, 4], f32, tag="ksc")  # g/|k| per sk
        rqs_sb = persist.tile([128, NT, 4], f32, tag="rqs")  # 1/|q| per sq
        v_sb = persist.tile([128, NT, 4, 65], bf16, tag="v")  # v + ones col
        nc.vector.memset(v_sb[:, :, :, 64], 1.0)
        # bf16 staging that persists so tiles 8-15's transposes can run
        # underneath the (Act-bound) attention phase
        vkna_sb = persist.tile([128, NT, 512], bf16, tag="vkna")
        qna_sb = persist.tile([128, NT, 4, 64], bf16, tag="qna")

        tp_ctx = ExitStack()
        ptp = tp_ctx.enter_context(
            tc.tile_pool(name=f"pt_ps{_rep}", bufs=2, space="PSUM"))
        proj_ctx = ExitStack()
        pnp = proj_ctx.enter_context(
            tc.tile_pool(name=f"pn_ps{_rep}", bufs=3, space="PSUM"))

        inv_g2 = 1.0 / (g * g)

        def finalize_k(st_i, on_act):
            """PE-transpose raw k tile into kt_sb."""
            kna = vkna_sb[:, st_i, 256:512].rearrange(
                "p (h d) -> p h d", h=4)
            for pair in range(2):
                pst = ptp.tile([128, 128], bf16, tag="t_ps", name="t_ps")
                nc.tensor.transpose(
                    pst[:, :], kna[:, 2 * pair:2 * pair + 2, :], ident[:, :])
                dstk = kt_sb[:, pair, st_i * 128:(st_i + 1) * 128]
                if on_act and pair == 0:
                    nc.scalar.copy(dstk, pst[:, :])
                else:
                    nc.vector.tensor_copy(dstk, pst[:, :])

        def finalize_q(st_i, on_act):
            """Normalize q tile (hat = qna * 1/|q|) and transpose into qt_sb."""
            hat = ptmp.tile([128, 4, 64], bf16, tag="hat", name="hat")
            for h4 in range(4):
                nc.vector.tensor_scalar_mul(
                    hat[:, h4, :], qna_sb[:, st_i, h4, :],
                    rqs_sb[:, st_i, h4:h4 + 1])
            for pair in range(2):
                pst = ptp.tile([128, 128], bf16, tag="t_ps", name="t_ps")
                nc.tensor.transpose(
                    pst[:, :], hat[:, 2 * pair:2 * pair + 2, :], ident[:, :])
                dstq = qt_sb[:, pair, st_i * 128:(st_i + 1) * 128]
                if on_act and pair == 0:
                    nc.scalar.copy(dstq, pst[:, :])
                else:
                    nc.vector.tensor_copy(dstq, pst[:, :])

        # ---- phase A: fused v+k projection; k stays RAW (g/|k| folded
        # into the exp scale later); k^T produced by PE transpose ----
        for st_i in range(NT):
            sl = slice(st_i * 128, (st_i + 1) * 128)
            psvk = pnp.tile([128, 512], f32, tag="nat", name="nat")
            for ec in range(NE):
                nc.tensor.matmul(
                    psvk[:, :], lhsT=x_sbs[ec][:, sl], rhs=wvk_sb[:, ec, :],
                    start=(ec == 0), stop=(ec == NE - 1))
            # one bf16 staging copy for v+k; v_sb filled from it (cheap
            # SBUF->SBUF copy), k half doubles as the transpose input
            nc.vector.tensor_copy(vkna_sb[:, st_i, :], psvk[:, :])
            nc.gpsimd.tensor_copy(
                v_sb[:, st_i, :, 0:64],
                vkna_sb[:, st_i, 0:256].rearrange("p (h d) -> p h d", h=4))
            kna = vkna_sb[:, st_i, 256:512].rearrange("p (h d) -> p h d", h=4)
            ksq = ptmp.tile([128, 4, 64], bf16, tag="ksq", name="ksq")
            nc.gpsimd.tensor_mul(ksq[:, :, :], kna[:, :, :], kna[:, :, :])
            ssk = ptmp.tile([128, 4], f32, tag="ssk", name="ssk")
            nc.vector.tensor_reduce(
                ssk[:, :], ksq[:, :, :], axis=mybir.AxisListType.X, op=ALU.add)
            rnk = ptmp.tile([128, 4], f32, tag="rnk", name="rnk")
            # |k|/g = sqrt(ssk/g^2); reciprocal -> g/|k|
            nc.scalar.activation(rnk[:, :], ssk[:, :], AF.Sqrt, scale=inv_g2)
            nc.vector.reciprocal(ksc_sb[:, st_i, :], rnk[:, :])
            if st_i < 8:
                finalize_k(st_i, on_act=True)

        # ---- phase B: q projection + l2 normalize + transpose ----
        for st_i in range(NT):
            sl = slice(st_i * 128, (st_i + 1) * 128)
            psq = pnp.tile([128, 4, 64], f32, tag="nat", name="nat")
            for ec in range(NE):
                nc.tensor.matmul(
                    psq[:, :, :], lhsT=ke_sbs[ec][:, sl], rhs=wq_sb[:, ec, :],
                    start=(ec == 0), stop=(ec == NE - 1))
            nc.scalar.copy(qna_sb[:, st_i, :, :], psq[:, :, :])
            qsq = ptmp.tile([128, 4, 64], bf16, tag="ksq", name="qsq")
            nc.gpsimd.tensor_mul(
                qsq[:, :, :], qna_sb[:, st_i, :, :], qna_sb[:, st_i, :, :])
            ssq = ptmp.tile([128, 4], f32, tag="ssk", name="ssq")
            nc.vector.tensor_reduce(
                ssq[:, :], qsq[:, :, :], axis=mybir.AxisListType.X, op=ALU.add)
            rnq = ptmp.tile([128, 4], f32, tag="rnk", name="rnq")
            nc.scalar.activation(rnq[:, :], ssq[:, :], AF.Sqrt)
            nc.vector.reciprocal(rqs_sb[:, st_i, :], rnq[:, :])
            if st_i < 8:
                finalize_q(st_i, on_act=True)
        proj_ctx.close()

        # ---- attention: one head at a time, transposed scores, ones-row
        # denominator; st double-buffered so exp (Act) overlaps matmuls (PE).
        # j OUTER: the j=0 round only needs tiles 0-7, so tiles 8-15's
        # transposes (PE/DVE-only) hide under the Act-bound j=0 round.
        with tc.tile_pool(name=f"st_ps{_rep}", bufs=2, space="PSUM") as stp, \
             tc.tile_pool(name=f"o_ps{_rep}", bufs=1, space="PSUM") as op:
            for j in range(NJ):
                for h in range(4):
                    pair, hp = h // 2, h % 2
                    psl = slice(hp * 64, (hp + 1) * 64)
                    ops = op.tile([65, SQB], f32, tag="o", name="o")
                    for i in range(8 * j + 8):
                        co = max(0, 128 * i - SQB * j)
                        corner = 128 * i >= SQB * j
                        ksl = slice(i * 128, (i + 1) * 128)
                        st = stp.tile([128, SQB], f32, tag="st", name="st")
                        for c0, c1 in ((co, 512), (max(co, 512), SQB)):
                            if c0 >= c1:
                                continue
                            nc.tensor.matmul(
                                st[:, c0:c1],
                                lhsT=kt_sb[psl, pair, ksl],
                                rhs=qt_sb[psl, pair, j * SQB + c0:j * SQB + c1],
                                start=True, stop=True)
                        et = ep.tile([128, SQB], bf16, tag="e", name="e")
                        nc.scalar.activation(
                            et[:, co:SQB], st[:, co:SQB], AF.Exp,
                            scale=ksc_sb[:, i, h:h + 1])
                        if corner:
                            nc.vector.tensor_mul(
                                et[:, co:co + 128], et[:, co:co + 128],
                                mask_sb[:, :])
                        for a, b in ((co, 512), (max(co, 512), SQB)):
                            if a >= b:
                                continue
                            # last matmul per PSUM bank: bank A's final write
                            # is tile i == 8j+3 (corner), bank B's i == 8j+7
                            is_last = ((i == 8 * j + 3) if b <= 512
                                       else (i == 8 * j + 7))
                            nc.tensor.matmul(
                                ops[:, a:b],
                                lhsT=v_sb[:, i, h, :],
                                rhs=et[:, a:b],
                                start=(i == 0), stop=is_last)
                    ot = osb.tile([65, SQB], bf16, tag="osb", name="osb")
                    if j == NJ - 1 and h == 3:
                        # final block: Act is idle after the last exp - split
                        # the copy across Act+DVE and DMA halves in parallel
                        nc.scalar.copy(ot[:, 0:512], ops[:, 0:512])
                        nc.vector.tensor_copy(ot[:, 512:SQB], ops[:, 512:SQB])
                        nc.sync.dma_start(
                            out_d[h, :, j * SQB:j * SQB + 512], ot[:, 0:512])
                        nc.sync.dma_start(
                            out_d[h, :, j * SQB + 512:(j + 1) * SQB],
                            ot[:, 512:SQB])
                    else:
                        nc.vector.tensor_copy(ot[:, :], ops[:, :])
                        nc.sync.dma_start(
                            out_d[h, :, j * SQB:(j + 1) * SQB], ot[:, :])
                    if j == 0:
                        # hide tiles 8-15's transposes under the Act-bound
                        # j=0 round (PE/DVE only - no act-table thrash)
                        for t in (8 + 2 * h, 9 + 2 * h):
                            finalize_k(t, on_act=False)
                            finalize_q(t, on_act=False)
        tp_ctx.close()
    nc.compile()
    return nc


_NC_CACHE = {}


def _get_nc(g: float):
    if g not in _NC_CACHE:
        _NC_CACHE[g] = _build(g)
    return _NC_CACHE[g]


def _numpy_fallback(x, k_embed, attn_mask, key_padding_mask, Wq, Wk, Wv, g_scale):
    def l2n(t):
        n = np.sqrt((t * t).sum(-1, keepdims=True))
        return t / np.maximum(n, 1e-12)
    q = (k_embed @ Wq.T).reshape(B, S, H, Dh).transpose(0, 2, 1, 3)
    k = (x @ Wk.T).reshape(B, S, H, Dh).transpose(0, 2, 1, 3)
    v = (x @ Wv.T).reshape(B, S, H, Dh).transpose(0, 2, 1, 3)
    q, k = l2n(q), l2n(k)
    s = float(g_scale) * np.einsum('bhqd,bhkd->bhqk', q, k)
    s = np.where(attn_mask[None, None], -np.inf, s)
    s = np.where(key_padding_mask[:, None, None, :], -np.inf, s)
    s = s - s.max(-1, keepdims=True)
    e = np.exp(s)
    a = e / e.sum(-1, keepdims=True)
    o = np.einsum('bhqk,bhkd->bhqd', a, v)
    return o.transpose(0, 2, 1, 3).reshape(B, S, E).astype(np.float32)


def _make_in_maps(x, k_embed, Wq, Wk, Wv):
    import ml_dtypes
    bf = ml_dtypes.bfloat16
    mask01 = np.triu(np.ones((128, 128), np.float32)).astype(bf)  # keep sq >= sk
    in_maps = []
    for c in range(8):
        b, hg = c // 2, c % 2
        rows = slice(hg * 256, (hg + 1) * 256)
        wv_t = Wv[rows].T                       # [512, 256]
        wk_t = Wk[rows].T
        in_maps.append({
            "xt": np.ascontiguousarray(x[b].T).astype(bf),
            "ket": np.ascontiguousarray(k_embed[b].T).astype(bf),
            "wqt": np.ascontiguousarray(Wq[rows].T).astype(bf),
            "wvkt": np.ascontiguousarray(
                np.concatenate([wv_t, wk_t], axis=1)).astype(bf),
            "mask01": mask01,
        })
    return in_maps


def kernel(**inputs) -> np.ndarray:
    x = np.asarray(inputs["x"], np.float32)
    k_embed = np.asarray(inputs["k_embed"], np.float32)
    attn_mask = np.asarray(inputs["attn_mask"])
    key_padding_mask = np.asarray(inputs["key_padding_mask"])
    Wq = np.asarray(inputs["Wq"], np.float32)
    Wk = np.asarray(inputs["Wk"], np.float32)
    Wv = np.asarray(inputs["Wv"], np.float32)
    g = float(np.asarray(inputs["g_scale"]))

    causal = np.triu(np.ones((S, S), bool), k=1)
    if (attn_mask != causal).any() or key_padding_mask.any():
        return _numpy_fallback(x, k_embed, attn_mask, key_padding_mask,
                               Wq, Wk, Wv, g)

    from concourse.bass_utils import run_bass_kernel_spmd

    nc = _get_nc(g)
    in_maps = _make_in_maps(x, k_embed, Wq, Wk, Wv)
    res = run_bass_kernel_spmd(nc, in_maps, core_ids=list(range(8)))
    kernel._last_results = res

    out = np.empty((B, S, E), np.float32)
    for c in range(8):
        b, hg = c // 2, c % 2
        r = np.asarray(res.results[c]["outt"], dtype=np.float32)  # [4, 65, S]
        w = r[:, :64, :] / r[:, 64:65, :]   # normalize by softmax denom
        for hh in range(4):
            h = hg * 4 + hh
            out[b, :, h * 64:(h + 1) * 64] = w[hh].T
    return out


# revision 24
# speedup vs baseline: 1.0151x; 1.0037x over previous
"""Trainium2 Bass kernel for causal cross-attention with L2-normalized q/k.

Reference computation (B=4, S=2048, E=512, H=8, Dh=64):
    q = k_embed @ Wq.T ; k = x @ Wk.T ; v = x @ Wv.T        (per batch)
    q,k l2-normalized over Dh per head; scores = g * q @ k.T (causal mask)
    out = softmax(scores) @ v

Sharding: 8 cores = 4 batches x 2 head-groups (4 heads each). Each core:
  - all matmul operands in bf16 (PE runs fp32 at 1/4 rate); inputs are
    host-pre-cast, accumulation stays fp32 in PSUM
  - k is NEVER normalized on-chip: scores use raw k^T and the factor
    g/|k_sk| is folded into the exp's per-partition scale (st has sk on
    the partition dim). The norm is computed from the same bf16 k the
    matmul consumes, so normalization is exact wrt quantized k.
  - q l2-normalized in natural layout (per-partition ops on SBUF bf16)
    then PE-transposed into [head_dim, seq] layout as bf16
  - scores computed transposed: st[sk, sq]; the softmax denominator comes
    from a ones-row appended to v; heads processed ONE at a time so the
    score PSUM tiles double-buffer (PE and Act engine overlap): per head
    st 2x[128,1024] (4 banks) + out accum 2x[65,1024] (4 banks) = 8 banks
  - PSUM->SBUF copies of k^T/q^T go on the Act engine (Copy lives in
    every act table - no table thrash); stat chains stay on DVE
  - returns outT [4 heads, 65, S] bf16 (64 rows of unnormalized out^T + the
    softmax denominator row); host divides and reassembles in fp32.
"""

import numpy as np

B, S, E, H = 4, 2048, 512, 8
Dh = 64
NE = E // 128          # 4 contraction chunks
NT = S // 128          # 16 s-tiles
SQB = 1024             # sq block width (2 PSUM banks)
NJ = S // SQB          # 2


def _build(g: float, repeats: int = 1):
    from contextlib import ExitStack

    import concourse.tile as tile
    from concourse import bacc, mybir
    from concourse.masks import make_identity

    f32 = mybir.dt.float32
    bf16 = mybir.dt.bfloat16
    AF = mybir.ActivationFunctionType
    ALU = mybir.AluOpType

    nc = bacc.Bacc("TRN2", target_bir_lowering=False, debug=False)
    xT_d = nc.dram_tensor("xt", [E, S], bf16, kind="ExternalInput")
    keT_d = nc.dram_tensor("ket", [E, S], bf16, kind="ExternalInput")
    wqT_d = nc.dram_tensor("wqt", [E, 256], bf16, kind="ExternalInput")
    wvkT_d = nc.dram_tensor("wvkt", [E, 512], bf16, kind="ExternalInput")
    mask_d = nc.dram_tensor("mask01", [128, 128], bf16, kind="ExternalInput")
    out_d = nc.dram_tensor("outt", [4, 65, S], bf16, kind="ExternalOutput")

    with tile.TileContext(nc) as tc:
     for _rep in range(repeats):
      with ExitStack() as ctx:
        persist = ctx.enter_context(tc.tile_pool(name=f"persist{_rep}", bufs=1))
        ptmp = ctx.enter_context(tc.tile_pool(name=f"ptmp{_rep}", bufs=3))
        ep = ctx.enter_context(tc.tile_pool(name=f"e_pool{_rep}", bufs=3))
        osb = ctx.enter_context(tc.tile_pool(name=f"o_sb{_rep}", bufs=2))

        # ---- input DMAs (x/ke split into halves for earlier start) ----
        wq_sb = persist.tile([128, NE, 256], bf16, tag="wq")
        wvk_sb = persist.tile([128, NE, 512], bf16, tag="wvk")
        mask_sb = persist.tile([128, 128], bf16, tag="mask")

        SQ4 = S // 4
        x_sbs, ke_sbs = [], []
        for ec in range(NE):
            t = persist.tile([128, S], bf16, tag=f"x{ec}", name=f"x{ec}")
            x_sbs.append(t)
        for ec in range(NE):
            t = persist.tile([128, S], bf16, tag=f"ke{ec}", name=f"ke{ec}")
            ke_sbs.append(t)
        # issue order = consumption order: wvk + x quarter-0 gate the first
        # projection; wq/mask/ke follow; later x/ke quarters last
        nc.sync.dma_start(out=wvk_sb[:, 0, :], in_=wvkT_d[0:128, :])
        for ec in range(NE):
            nc.sync.dma_start(
                out=x_sbs[ec][:, 0:SQ4],
                in_=xT_d[ec * 128:(ec + 1) * 128, 0:SQ4])
        nc.sync.dma_start(
            out=wvk_sb[:, 1:NE, :],
            in_=wvkT_d[128:].rearrange("(c p) m -> p c m", p=128))
        nc.sync.dma_start(out=wq_sb, in_=wqT_d.rearrange("(c p) m -> p c m", p=128))
        nc.sync.dma_start(out=mask_sb, in_=mask_d[:, :])
        for qf in range(1, 4):
            for ec in range(NE):
                nc.sync.dma_start(
                    out=x_sbs[ec][:, qf * SQ4:(qf + 1) * SQ4],
                    in_=xT_d[ec * 128:(ec + 1) * 128, qf * SQ4:(qf + 1) * SQ4])
        for qf in range(4):
            for ec in range(NE):
                nc.sync.dma_start(
                    out=ke_sbs[ec][:, qf * SQ4:(qf + 1) * SQ4],
                    in_=keT_d[ec * 128:(ec + 1) * 128, qf * SQ4:(qf + 1) * SQ4])

        ident = persist.tile([128, 128], bf16, tag="ident")
        make_identity(nc, ident[:, :])

        qt_sb = persist.tile([128, 2, S], bf16, tag="qt")   # qhat^T per pair
        kt_sb = persist.tile([128, 2, S], bf16, tag="kt")   # raw k^T per pair
        ksc_sb = persist.tile([128, NT, 4], f32, tag="ksc")  # g/|k| per sk
        rqs_sb = persist.tile([128, NT, 4], f32, tag="rqs")  # 1/|q| per sq
        v_sb = persist.tile([128, NT, 4, 65], bf16, tag="v")  # v + ones col
        nc.vector.memset(v_sb[:, :, :, 64], 1.0)
        # bf16 staging that persists so tiles 8-15's transposes can run
        # underneath the (Act-bound) attention phase
        vkna_sb = persist.tile([128, NT, 512], bf16, tag="vkna")
        qna_sb = persist.tile([128, NT, 4, 64], bf16, tag="qna")

        tp_ctx = ExitStack()
        ptp = tp_ctx.enter_context(
            tc.tile_pool(name=f"pt_ps{_rep}", bufs=2, space="PSUM"))
        proj_ctx = ExitStack()
        pnp = proj_ctx.enter_context(
            tc.tile_pool(name=f"pn_ps{_rep}", bufs=3, space="PSUM"))

        inv_g2 = 1.0 / (g * g)

        def finalize_k(st_i, on_act):
            """PE-transpose raw k tile into kt_sb."""
            kna = vkna_sb[:, st_i, 256:512].rearrange(
                "p (h d) -> p h d", h=4)
            for pair in range(2):
                pst = ptp.tile([128, 128], bf16, tag="t_ps", name="t_ps")
                nc.tensor.transpose(
                    pst[:, :], kna[:, 2 * pair:2 * pair + 2, :], ident[:, :])
                dstk = kt_sb[:, pair, st_i * 128:(st_i + 1) * 128]
                if on_act and pair == 0:
                    nc.scalar.copy(dstk, pst[:, :])
                else:
                    nc.vector.tensor_copy(dstk, pst[:, :])

        def finalize_q(st_i, on_act):
            """Normalize q tile (hat = qna * 1/|q|) and transpose into qt_sb."""
            hat = ptmp.tile([128, 4, 64], bf16, tag="hat", name="hat")
            for h4 in range(4):
                nc.vector.tensor_scalar_mul(
                    hat[:, h4, :], qna_sb[:, st_i, h4, :],
                    rqs_sb[:, st_i, h4:h4 + 1])
            for pair in range(2):
                pst = ptp.tile([128, 128], bf16, tag="t_ps", name="t_ps")
                nc.tensor.transpose(
                    pst[:, :], hat[:, 2 * pair:2 * pair + 2, :], ident[:, :])
                dstq = qt_sb[:, pair, st_i * 128:(st_i + 1) * 128]
                if on_act and pair == 0:
                    nc.scalar.copy(dstq, pst[:, :])
                else:
                    nc.vector.tensor_copy(dstq, pst[:, :])

        # ---- phase A: fused v+k projection; k stays RAW (g/|k| folded
        # into the exp scale later); k^T produced by PE transpose ----
        for st_i in range(NT):
            sl = slice(st_i * 128, (st_i + 1) * 128)
            psvk = pnp.tile([128, 512], f32, tag="nat", name="nat")
            for ec in range(NE):
                nc.tensor.matmul(
                    psvk[:, :], lhsT=x_sbs[ec][:, sl], rhs=wvk_sb[:, ec, :],
                    start=(ec == 0), stop=(ec == NE - 1))
            # one bf16 staging copy for v+k; v_sb filled from it (cheap
            # SBUF->SBUF copy), k half doubles as the transpose input
            nc.vector.tensor_copy(vkna_sb[:, st_i, :], psvk[:, :])
            nc.gpsimd.tensor_copy(
                v_sb[:, st_i, :, 0:64],
                vkna_sb[:, st_i, 0:256].rearrange("p (h d) -> p h d", h=4))
            kna = vkna_sb[:, st_i, 256:512].rearrange("p (h d) -> p h d", h=4)
            ksq = ptmp.tile([128, 4, 64], bf16, tag="ksq", name="ksq")
            nc.gpsimd.tensor_mul(ksq[:, :, :], kna[:, :, :], kna[:, :, :])
            ssk = ptmp.tile([128, 4], f32, tag="ssk", name="ssk")
            nc.vector.tensor_reduce(
                ssk[:, :], ksq[:, :, :], axis=mybir.AxisListType.X, op=ALU.add)
            rnk = ptmp.tile([128, 4], f32, tag="rnk", name="rnk")
            # |k|/g = sqrt(ssk/g^2); reciprocal -> g/|k|
            nc.scalar.activation(rnk[:, :], ssk[:, :], AF.Sqrt, scale=inv_g2)
            nc.vector.reciprocal(ksc_sb[:, st_i, :], rnk[:, :])
            if st_i < 8:
                finalize_k(st_i, on_act=True)

        # ---- phase B: q projection + l2 normalize + transpose ----
        for st_i in range(NT):
            sl = slice(st_i * 128, (st_i + 1) * 128)
            psq = pnp.tile([128, 4, 64], f32, tag="nat", name="nat")
            for ec in range(NE):
                nc.tensor.matmul(
                    psq[:, :, :], lhsT=ke_sbs[ec][:, sl], rhs=wq_sb[:, ec, :],
                    start=(ec == 0), stop=(ec == NE - 1))
            nc.scalar.copy(qna_sb[:, st_i, :, :], psq[:, :, :])
            qsq = ptmp.tile([128, 4, 64], bf16, tag="ksq", name="qsq")
            nc.gpsimd.tensor_mul(
                qsq[:, :, :], qna_sb[:, st_i, :, :], qna_sb[:, st_i, :, :])
            ssq = ptmp.tile([128, 4], f32, tag="ssk", name="ssq")
            nc.vector.tensor_reduce(
                ssq[:, :], qsq[:, :, :], axis=mybir.AxisListType.X, op=ALU.add)
            rnq = ptmp.tile([128, 4], f32, tag="rnk", name="rnq")
            nc.scalar.activation(rnq[:, :], ssq[:, :], AF.Sqrt)
            nc.vector.reciprocal(rqs_sb[:, st_i, :], rnq[:, :])
            if st_i < 8:
                finalize_q(st_i, on_act=True)
        proj_ctx.close()

        # ---- attention: one head at a time, transposed scores, ones-row
        # denominator; st double-buffered so exp (Act) overlaps matmuls (PE).
        # j OUTER: the j=0 round only needs tiles 0-7, so tiles 8-15's
        # transposes (PE/DVE-only) hide under the Act-bound j=0 round.
        with tc.tile_pool(name=f"st_ps{_rep}", bufs=2, space="PSUM") as stp, \
             tc.tile_pool(name=f"o_ps{_rep}", bufs=1, space="PSUM") as op:
            for j in range(NJ):
                for h in range(4):
                    pair, hp = h // 2, h % 2
                    psl = slice(hp * 64, (hp + 1) * 64)
                    ops = op.tile([65, SQB], f32, tag="o", name="o")
                    for i in range(8 * j + 8):
                        co = max(0, 128 * i - SQB * j)
                        corner = 128 * i >= SQB * j
                        ksl = slice(i * 128, (i + 1) * 128)
                        st = stp.tile([128, SQB], f32, tag="st", name="st")
                        for c0, c1 in ((co, 512), (max(co, 512), SQB)):
                            if c0 >= c1:
                                continue
                            nc.tensor.matmul(
                                st[:, c0:c1],
                                lhsT=kt_sb[psl, pair, ksl],
                                rhs=qt_sb[psl, pair, j * SQB + c0:j * SQB + c1],
                                start=True, stop=True)
                        et = ep.tile([128, SQB], bf16, tag="e", name="e")
                        nc.scalar.activation(
                            et[:, co:SQB], st[:, co:SQB], AF.Exp,
                            scale=ksc_sb[:, i, h:h + 1])
                        if corner:
                            nc.vector.tensor_mul(
                                et[:, co:co + 128], et[:, co:co + 128],
                                mask_sb[:, :])
                        for a, b in ((co, 512), (max(co, 512), SQB)):
                            if a >= b:
                                continue
                            # last matmul per PSUM bank: bank A's final write
                            # is tile i == 8j+3 (corner), bank B's i == 8j+7
                            is_last = ((i == 8 * j + 3) if b <= 512
                                       else (i == 8 * j + 7))
                            nc.tensor.matmul(
                                ops[:, a:b],
                                lhsT=v_sb[:, i, h, :],
                                rhs=et[:, a:b],
                                start=(i == 0), stop=is_last)
                    ot = osb.tile([65, SQB], bf16, tag="osb", name="osb")
                    if j == NJ - 1 and h == 3:
                        # final block: Act is idle after the last exp - split
                        # the copy across Act+DVE and DMA halves in parallel
                        nc.scalar.copy(ot[:, 0:512], ops[:, 0:512])
                        nc.vector.tensor_copy(ot[:, 512:SQB], ops[:, 512:SQB])
                        nc.sync.dma_start(
                            out_d[h, :, j * SQB:j * SQB + 512], ot[:, 0:512])
                        nc.sync.dma_start(
                            out_d[h, :, j * SQB + 512:(j + 1) * SQB],
                            ot[:, 512:SQB])
                    else:
                        nc.vector.tensor_copy(ot[:, :], ops[:, :])
                        nc.sync.dma_start(
                            out_d[h, :, j * SQB:(j + 1) * SQB], ot[:, :])
                    if j == 0:
                        # hide tiles 8-15's transposes under the Act-bound
                        # j=0 round (PE/DVE only - no act-table thrash)
                        for t in (8 + 2 * h, 9 + 2 * h):
                            finalize_k(t, on_act=False)
                            finalize_q(t, on_act=False)
        tp_ctx.close()
    nc.compile()
    return nc


_NC_CACHE = {}


def _get_nc(g: float):
    if g not in _NC_CACHE:
        _NC_CACHE[g] = _build(g)
    return _NC_CACHE[g]


def _numpy_fallback(x, k_embed, attn_mask, key_padding_mask, Wq, Wk, Wv, g_scale):
    def l2n(t):
        n = np.sqrt((t * t).sum(-1, keepdims=True))
        return t / np.maximum(n, 1e-12)
    q = (k_embed @ Wq.T).reshape(B, S, H, Dh).transpose(0, 2, 1, 3)
    k = (x @ Wk.T).reshape(B, S, H, Dh).transpose(0, 2, 1, 3)
    v = (x @ Wv.T).reshape(B, S, H, Dh).transpose(0, 2, 1, 3)
    q, k = l2n(q), l2n(k)
    s = float(g_scale) * np.einsum('bhqd,bhkd->bhqk', q, k)
    s = np.where(attn_mask[None, None], -np.inf, s)
    s = np.where(key_padding_mask[:, None, None, :], -np.inf, s)
    s = s - s.max(-1, keepdims=True)
    e = np.exp(s)
    a = e / e.sum(-1, keepdims=True)
    o = np.einsum('bhqk,bhkd->bhqd', a, v)
    return o.transpose(0, 2, 1, 3).reshape(B, S, E).astype(np.float32)


def _make_in_maps(x, k_embed, Wq, Wk, Wv):
    import ml_dtypes
    bf = ml_dtypes.bfloat16
    mask01 = np.triu(np.ones((128, 128), np.float32)).astype(bf)  # keep sq >= sk
    in_maps = []
    for c in range(8):
        b, hg = c // 2, c % 2
        rows = slice(hg * 256, (hg + 1) * 256)
        wv_t = Wv[rows].T                       # [512, 256]
        wk_t = Wk[rows].T
        in_maps.append({
            "xt": np.ascontiguousarray(x[b].T).astype(bf),
            "ket": np.ascontiguousarray(k_embed[b].T).astype(bf),
            "wqt": np.ascontiguousarray(Wq[rows].T).astype(bf),
            "wvkt": np.ascontiguousarray(
                np.concatenate([wv_t, wk_t], axis=1)).astype(bf),
            "mask01": mask01,
        })
    return in_maps


def kernel(**inputs) -> np.ndarray:
    x = np.asarray(inputs["x"], np.float32)
    k_embed = np.asarray(inputs["k_embed"], np.float32)
    attn_mask = np.asarray(inputs["attn_mask"])
    key_padding_mask = np.asarray(inputs["key_padding_mask"])
    Wq = np.asarray(inputs["Wq"], np.float32)
    Wk = np.asarray(inputs["Wk"], np.float32)
    Wv = np.asarray(inputs["Wv"], np.float32)
    g = float(np.asarray(inputs["g_scale"]))

    causal = np.triu(np.ones((S, S), bool), k=1)
    if (attn_mask != causal).any() or key_padding_mask.any():
        return _numpy_fallback(x, k_embed, attn_mask, key_padding_mask,
                               Wq, Wk, Wv, g)

    from concourse.bass_utils import run_bass_kernel_spmd

    nc = _get_nc(g)
    in_maps = _make_in_maps(x, k_embed, Wq, Wk, Wv)
    res = run_bass_kernel_spmd(nc, in_maps, core_ids=list(range(8)))
    kernel._last_results = res

    out = np.empty((B, S, E), np.float32)
    for c in range(8):
        b, hg = c // 2, c % 2
        r = np.asarray(res.results[c]["outt"], dtype=np.float32)  # [4, 65, S]
        w = r[:, :64, :] / r[:, 64:65, :]   # normalize by softmax denom
        for hh in range(4):
            h = hg * 4 + hh
            out[b, :, h * 64:(h + 1) * 64] = w[hh].T
    return out
